# revision 20
# baseline (speedup 1.0000x reference)
"""Self-contained 8-core Trainium2 Bass kernel for MultiHeadAttention.

Problem: B=2, S=2048, D=1024, H=16 heads (hd=64), f32, self-attention
(no mask), eval mode (dropout = identity).

Sharding: data-parallel over B (2) x tensor-parallel over heads (4 groups
of 4 heads) = 8 cores. Each core computes, for its batch b and its 4
heads: Q/K/V projections (column-sliced), attention, and a partial
output projection (row-sliced Wo). Host sums the 4 partials per batch
and adds the (bv @ Wo + bo) correction (bv never enters the kernel:
ctx rows sum probs to 1, so (ctx+bv) @ Wo = ctx @ Wo + bv @ Wo).

Algebraic simplifications (exact):
  - bk dropped: softmax over k is invariant to the per-q constant Q.bk.
  - softmax without max subtraction (scores bounded, exp safe in f32).
  - bq added per-partition (feature) to Q^T after projection.
  - row normalization deferred past P@V (scale ctx, not probs); row sums
    obtained free via an appended ones-column in V.

v3 changes vs v2 (235.6us harness / 246.9us local):
  - scores row-tiled: the two heads of a pair run as CONCURRENT K=64
    matmuls on PE row-groups (0,0)/(64,0) (auto tile_position from the
    sliced APs) instead of two serial full-array masked matmuls. Halves
    the scores streaming time; the masked-Q/bias trick is gone (plain
    per-partition tensor_scalar_add of bq).
  - norm chain restructured: both heads' denominators are copied into
    one [2, 512] tile and reciprocal'd in ONE DVE call (reciprocal cost
    is per-lane-element, so [2,512] costs the same as [1,512]: was 2x
    3.3us, now 1x), then two gpsimd partition_broadcasts.
  - head-of-line fix: outproj fillers are injected only from r>=8 of the
    next attention stretch and the deferred norm muls at r==5, so the
    in-order PE queue never parks on the ~5us norm-chain latency (the
    v2 trace showed four ~6.3us PE stalls + HAM re-throttle cycles from
    outproj MMs queued at r==2 waiting on reciprocal->broadcast->mul).
  - qt1 slabs 1-3 deferred into the pair-1 qc0 attention stretch (which
    previously had no filler work and idled the PE ~6us); kt1 + qt1(s0)
    remain fillers for the pair-0 qc1-3 stretches.
  - tail: last-qc normalization in 128-column chunks (reciprocal [2,128]
    = 0.9us latency instead of 3.3) with outproj units emitted per chunk.

Layouts per core: x^T [D, S] slabs; K^T/Q^T per head-pair with the two
heads stacked on partitions (64 features each); scores^T computed per
head as K-chunk[64f, 128k] @ Q^T[64f, 512q] with k-positions on output
partitions, both heads concurrent via PE row tiling; exp on ACT (f32
PSUM -> bf16 SBUF, [128, 1024] covering both heads); PV accumulates
ctx^T[hd+1, q] over r in PSUM with the ones-row giving the softmax
denominator; out projection contracts head dims with Wo as the moving
operand.
"""

import sys

sys.path.insert(0, "/opt/trn_rl_repo")

import numpy as np
import ml_dtypes

B, S, D, H, HD = 2, 2048, 1024, 16, 64
HPC = 4  # heads per core
NCORES = 8
DC = D // 128  # 8 contraction chunks
ST = S // 128  # 16 s-tiles
QCW = 512  # q chunk width == slab width
QC = S // QCW  # 4 q chunks == 4 slabs
KT = S // 128  # 16 k tiles

_CACHE = {}


def _build(repeat=1, warmup=8):
    import concourse.bass as bass  # noqa: F401
    import concourse.mybir as mybir
    import concourse.tile as tile
    from concourse import bacc
    from concourse.alu_op_type import AluOpType
    from concourse.library_config import attn as attn_lib

    # The act-table-load pass assigns each activation the FIRST table set
    # containing its function, so a kernel using both Exp and Ln thrashes
    # between exp_and_others and natural_log (2.7us per switch, twice per
    # attention stretch). Strip Exp/Ln from every set except the combined
    # natural_log_exp_and_others (order/indices preserved, so emitted
    # act_func_set_ids still match act_info.json) so both functions
    # resolve to one resident set.
    if not getattr(bacc, "_ln_exp_tables_patched", False):
        _orig_gat = bacc.get_activation_tables

        def _gat(module_arch):
            t = _orig_gat(module_arch)
            AFt = mybir.ActivationFunctionType
            for name, fns in t.items():
                if name != "natural_log_exp_and_others":
                    fns.discard(AFt.Exp)
                    fns.discard(AFt.Ln)
                    fns.discard(AFt.Copy)
            return t

        bacc.get_activation_tables = _gat
        bacc._ln_exp_tables_patched = True

    F32 = mybir.dt.float32
    BF16 = mybir.dt.bfloat16
    AF = mybir.ActivationFunctionType

    nc = bacc.Bacc("TRN2", target_bir_lowering=False, debug=False)

    xt_d = nc.dram_tensor("xt", [D, S], BF16, kind="ExternalInput")
    wq_d = nc.dram_tensor("wq", [D, HPC * HD], BF16, kind="ExternalInput")
    wk_d = nc.dram_tensor("wk", [D, HPC * HD], BF16, kind="ExternalInput")
    wv_d = nc.dram_tensor("wv", [D, HPC * HD], BF16, kind="ExternalInput")
    wo_d = nc.dram_tensor("wo", [HPC * HD, D], BF16, kind="ExternalInput")
    bqm_d = nc.dram_tensor("bqm2", [128, 2], F32, kind="ExternalInput")
    out_d = nc.dram_tensor("out_p", [S, D], BF16, kind="ExternalOutput")

    with tile.TileContext(nc) as tc:
        nc.gpsimd.load_library(attn_lib)
        with (
            tc.tile_pool(name="wp", bufs=1) as wp,
            tc.tile_pool(name="xp", bufs=1) as xp,
            tc.tile_pool(name="qk", bufs=1) as qk,
            tc.tile_pool(name="vp", bufs=1) as vp,
            tc.tile_pool(name="ep", bufs=14) as ep,
            tc.tile_pool(name="cp", bufs=1) as cp,
            tc.tile_pool(name="c2", bufs=4) as c2p,
            tc.tile_pool(name="mp", bufs=2) as mp,
            tc.tile_pool(name="op", bufs=4) as op,
            tc.tile_pool(name="pp", bufs=2, space="PSUM") as pp,
        ):
            # persistent tiles (loaded / initialized once, reused each rep)
            wk_t = wp.tile([128, DC, HPC * HD], BF16, tag="wk")
            wq_t = wp.tile([128, DC, HPC * HD], BF16, tag="wq")
            wv_t = wp.tile([128, DC, HPC * HD], BF16, tag="wv")
            wo_t = wp.tile([128, 2, D], BF16, tag="wo")
            bqm_t = wp.tile([128, 2], F32, tag="bqm")
            ones_b = wp.tile([128, 64], BF16, tag="ones")
            warm_in = wp.tile([128, QCW], BF16, tag="warmin")
            xt_t = xp.tile([128, DC, S], BF16, tag="xt")

            # DMA order: wk+wq then slab0 gate the first projections.
            # Weights ride the ACT HWDGE queue (idle at start) so they don't
            # serialize behind the x^T slab stream on the SP queue.
            nc.scalar.dma_start(wk_t[:], wk_d.rearrange("(c p) n -> p c n", p=128))
            nc.scalar.dma_start(wq_t[:], wq_d.rearrange("(c p) n -> p c n", p=128))
            nc.scalar.dma_start(bqm_t[:], bqm_d[:])
            nc.vector.memset(ones_b[:], 1.0)
            nc.vector.memset(warm_in[:], 0.0)

            import contextlib

            def emit_input_dmas():
                # ---- x^T slab DMAs (+ wv after slab0, wo after slab2)
                for s in range(QC):
                    qs = slice(s * QCW, (s + 1) * QCW)
                    for c in range(DC):
                        nc.sync.dma_start(
                            xt_t[:, c, qs], xt_d[c * 128:(c + 1) * 128, qs]
                        )
                    if s == 0:
                        nc.scalar.dma_start(
                            wv_t[:], wv_d.rearrange("(c p) n -> p c n", p=128)
                        )
                    elif s == 2:
                        nc.scalar.dma_start(
                            wo_t[:], wo_d.rearrange("(c p) n -> p c n", p=128)
                        )

            if repeat > 1:
                # In-loop DMAs go through software descriptor generation and
                # dominate the loop body, so hoist them for timing builds.
                emit_input_dmas()
                _engs = [mybir.EngineType.PE, mybir.EngineType.Activation,
                         mybir.EngineType.DVE, mybir.EngineType.SP,
                         mybir.EngineType.Pool]
                rep_ctx = tc.For_i(0, repeat, hint_engines=_engs, staggered_reset=True)
            else:
                rep_ctx = contextlib.nullcontext()
            with rep_ctx:
                if repeat == 1:
                    emit_input_dmas()

                # ---- per-rep tiles
                v1_t = vp.tile([128, ST, HPC * 65], BF16, tag="v1")
                with nc.allow_low_precision(reason="bf16 operands"):
                    nc.vector.tensor_copy(
                        v1_t[:].rearrange("p s (h c) -> p s h c", c=65)[:, :, :, 64],
                        ones_b[:, 0:64].rearrange("p (s h) -> p s h", s=ST),
                    )
                kt_t = [qk.tile([128, S], BF16, tag=f"kt{p}", name=f"kt{p}") for p in range(2)]
                qt_t = [qk.tile([128, QC, QCW], BF16, tag=f"qt{p}", name=f"qt{p}") for p in range(2)]
                ctxt_t = [cp.tile([128, S], BF16, tag=f"ct{p}", name=f"ct{p}") for p in range(2)]

                # ---- PE warmup: ramp the p-state before real work arrives.
                for w in range(warmup):
                    wps = pp.tile([64, QCW], F32, tag="proj", bufs=2, name="warm")
                    nc.tensor.matmul(
                        wps[:], ones_b[:, 0:64],
                        warm_in[:], start=True, stop=True,
                    )

                # ---- projection units
                def kt_proj_a(pair, s, state):
                    qs = slice(s * QCW, (s + 1) * QCW)
                    kps = pp.tile([128, QCW], F32, tag="proj", bufs=2, name="kps")
                    state.append(kps)
                    for c in range(DC // 2):
                        nc.tensor.matmul(
                            kps[:],
                            wk_t[:, c, pair * 128:(pair + 1) * 128],
                            xt_t[:, c, qs],
                            start=(c == 0),
                            stop=False,
                        )

                def kt_proj_b(pair, s, state):
                    qs = slice(s * QCW, (s + 1) * QCW)
                    kps = state.pop()
                    for c in range(DC // 2, DC):
                        nc.tensor.matmul(
                            kps[:],
                            wk_t[:, c, pair * 128:(pair + 1) * 128],
                            xt_t[:, c, qs],
                            start=False,
                            stop=(c == DC - 1),
                        )
                    with nc.allow_low_precision(reason="bf16 operands"):
                        nc.vector.tensor_copy(kt_t[pair][:, qs], kps[:])

                def kt_proj(pair, s):
                    st = []
                    kt_proj_a(pair, s, st)
                    kt_proj_b(pair, s, st)

                def qt_proj_a(pair, s, state):
                    qs = slice(s * QCW, (s + 1) * QCW)
                    qps = pp.tile([128, QCW], F32, tag="proj", bufs=2, name="qps")
                    state.append(qps)
                    for c in range(DC // 2):
                        nc.tensor.matmul(
                            qps[:],
                            wq_t[:, c, pair * 128:(pair + 1) * 128],
                            xt_t[:, c, qs],
                            start=(c == 0),
                            stop=False,
                        )

                def qt_proj_b(pair, s, state):
                    qs = slice(s * QCW, (s + 1) * QCW)
                    qps = state.pop()
                    for c in range(DC // 2, DC):
                        nc.tensor.matmul(
                            qps[:],
                            wq_t[:, c, pair * 128:(pair + 1) * 128],
                            xt_t[:, c, qs],
                            start=False,
                            stop=(c == DC - 1),
                        )
                    # bq added per-partition (feature dim) — heads need no
                    # masking since scores contract only their own 64 rows.
                    with nc.allow_low_precision(reason="bf16 operands"):
                        nc.vector.tensor_scalar_add(
                            qt_t[pair][:, s, :], qps[:],
                            bqm_t[:, pair:pair + 1],
                        )

                def qt_proj(pair, s):
                    st = []
                    qt_proj_a(pair, s, st)
                    qt_proj_b(pair, s, st)

                def v_proj(st):
                    vps = pp.tile([128, HPC * HD], F32, tag="proj", bufs=2, name="vps")
                    for c in range(DC):
                        nc.tensor.matmul(
                            vps[:],
                            xt_t[:, c, st * 128:(st + 1) * 128],
                            wv_t[:, c, :],
                            start=(c == 0),
                            stop=(c == DC - 1),
                        )
                    with nc.allow_low_precision(reason="bf16 operands"):
                        nc.vector.tensor_copy(
                            v1_t[:, st, :].rearrange("p (h c) -> p h c", c=65)[:, :, 0:64],
                            vps[:].rearrange("p (h c) -> p h c", c=64),
                        )

                # ---- attention pieces
                def pv_emit(pair, ctx_ps, r, expt):
                    for h in range(2):
                        hh = 2 * pair + h
                        nc.tensor.matmul(
                            ctx_ps[h][:],
                            v1_t[:, r, 65 * hh:65 * hh + 65],
                            expt[:, h * QCW:(h + 1) * QCW],
                            start=(r == 0),
                            stop=(r == KT - 1),
                        )

                def attn_rs(pair, qc, rs, ctx_ps, fillers=None, fill_at=(),
                            pre=None, pre_at=7, pend=None, flush=True,
                            collect=None):
                    # `pend` carries the not-yet-emitted PV of the previous r
                    # (possibly across the slab-window segments of one call).
                    # `collect` (a list) switches to scores+exp only: the
                    # (r, expt) pairs are stashed so their PVs can run later
                    # as fillers (used to thin the slab-window phase).
                    # `pre` is a list of deferred normalization-mul handle
                    # sets; one is consumed per r starting at n == pre_at.
                    fill_at = list(fill_at)
                    pre = list(pre) if pre else []
                    n = 0
                    for r in rs:
                        sreg = pp.tile([128, 2 * QCW], F32, tag="sreg", bufs=2)
                        expt = ep.tile([128, 2 * QCW], BF16, tag="exp")
                        # scores for the two heads run CONCURRENTLY as K=64
                        # row-group matmuls (tile_position auto-derived from
                        # the 0/64 base partitions of the sliced operands).
                        for h in range(2):
                            hs = slice(64 * h, 64 * (h + 1))
                            nc.tensor.matmul(
                                sreg[:, h * QCW:(h + 1) * QCW],
                                kt_t[pair][hs, r * 128:(r + 1) * 128],
                                qt_t[pair][hs, qc, :],
                                start=True,
                                stop=True,
                            )
                        with nc.allow_low_precision(reason="bf16 exp output"):
                            nc.scalar.activation(expt[:], sreg[:], AF.Exp, scale=0.125)
                        if collect is not None:
                            collect.append((r, expt))
                        else:
                            # emit the PREVIOUS r's PV only now: the next
                            # scores matmul never sits behind a PV that is
                            # waiting on exp, so exp is never starved.
                            if pend is not None:
                                pv_emit(pair, ctx_ps, *pend)
                            pend = (r, expt)
                        n += 1
                        if pre and pre_at <= n < pre_at + len(pre) + 16:
                            if n >= pre_at:
                                norm_b(*pre.pop(0))
                        while fillers and fill_at and n == fill_at[0]:
                            fill_at.pop(0)
                            fillers.pop(0)()
                    if flush and pend is not None:
                        pv_emit(pair, ctx_ps, *pend)
                        pend = None
                    return pend

                def norm_a(pair, qc, ctx_ps):
                    # Drain the ctx PSUM banks FIRST (they have no double
                    # buffer — the next stretch's PV group waits on their
                    # release), then gather both heads' denominators onto
                    # partitions 0 and 32 of one tile (engine partition
                    # bases must be 32-aligned) and reciprocal them in ONE
                    # call via exp(-ln(x)) on ACT.
                    handles = []
                    d2 = mp.tile([33, QCW], F32, tag="d2")
                    for h in range(2):
                        c2 = c2p.tile([65, QCW], F32, tag="c2s", name=f"c2s{h}")
                        nc.vector.tensor_copy(c2[:], ctx_ps[h][:])
                        nc.vector.tensor_copy(
                            d2[32 * h:32 * h + 1, :], c2[64:65, :]
                        )
                        handles.append(c2)
                    # reciprocal via exp(-ln(x)) on ACT: Ln and Exp share
                    # one table set (natural_log_exp_and_others), the two
                    # calls cost ~1.4us of otherwise-idle boundary ACT time,
                    # and the DVE FIFO is never blocked by the 3.3us DVE
                    # reciprocal (which delayed outproj PSUM drains).
                    lg = mp.tile([33, QCW], F32, tag="lg")
                    nc.scalar.activation(lg[:], d2[:], AF.Ln)
                    dr = mp.tile([33, QCW], F32, tag="dr")
                    nc.scalar.activation(dr[:], lg[:], AF.Exp, scale=-1.0)
                    r0s = []
                    for h in range(2):
                        r0 = mp.tile([1, QCW], F32, tag="rsum")
                        nc.vector.tensor_copy(r0[:], dr[32 * h:32 * h + 1, :])
                        r0s.append(r0)
                    out = []
                    for h in range(2):
                        bct = mp.tile([64, QCW], F32, tag="bc")
                        nc.gpsimd.partition_broadcast(bct[:], r0s[h][:])
                        out.append((handles[h], bct))
                    return (pair, qc, out)

                def norm_b(pair, qc, handles):
                    qs = slice(qc * QCW, (qc + 1) * QCW)
                    for h in range(2):
                        c2, bct = handles[h]
                        with nc.allow_low_precision(reason="bf16 ctx"):
                            nc.vector.tensor_mul(
                                ctxt_t[pair][64 * h:64 * (h + 1), qs],
                                c2[0:64, :],
                                bct[:],
                            )

                def outproj_unit(qc, sub, dcol, tail=False, ptag="proj"):
                    q0 = qc * QCW + sub * 128
                    ops = pp.tile([128, QCW], F32, tag=ptag, bufs=2, name="ops")[:]
                    for pair in range(2):
                        nc.tensor.matmul(
                            ops,
                            ctxt_t[pair][:, q0:q0 + 128],
                            wo_t[:, pair, dcol * 512:(dcol + 1) * 512],
                            start=(pair == 0),
                            stop=(pair == 1),
                        )
                    osb = op.tile([128, QCW], BF16, tag="osb")
                    with nc.allow_low_precision(reason="bf16 out"):
                        if tail and dcol == 1:
                            # ACT is idle in the tail: alternating the PSUM
                            # drain between DVE and ACT halves the cast pace
                            # that gates the 2-slot proj-PSUM ping-pong.
                            # (AF Copy lives in every table set: no switch.)
                            nc.scalar.activation(osb[:], ops, AF.Copy)
                        else:
                            nc.vector.tensor_copy(osb[:], ops)
                    eng = nc.scalar if tail else nc.sync
                    eng.dma_start(
                        out_d[q0:q0 + 128, dcol * 512:(dcol + 1) * 512], osb[:]
                    )

                def tail_norm_outproj(pair, qc, ctx_ps, extra=(), units=None):
                    # last-call variant: one batched norm chain over all 512
                    # columns (denominators straight from PSUM, reciprocal
                    # on ACT which is idle by now), with held-back outproj
                    # units as PE filler while the chain resolves, then all
                    # 8 output-projection units. Tail DMAs ride the ACT
                    # HWDGE queue (idle) to halve the final drain.
                    extra = list(extra)
                    d2c = mp.tile([33, QCW], F32, tag="d2t")
                    for h in range(2):
                        nc.vector.tensor_copy(
                            d2c[32 * h:32 * h + 1, :], ctx_ps[h][64:65, :]
                        )
                    lgc = mp.tile([33, QCW], F32, tag="lgt")
                    nc.scalar.activation(lgc[:], d2c[:], AF.Ln)
                    drc = mp.tile([33, QCW], F32, tag="drt")
                    nc.scalar.activation(drc[:], lgc[:], AF.Exp, scale=-1.0)
                    while extra:  # PE filler while the chain resolves
                        extra.pop(0)()
                    bcts = []
                    for h in range(2):
                        r0 = mp.tile([1, QCW], F32, tag="rsumt")
                        nc.vector.tensor_copy(r0[:], drc[32 * h:32 * h + 1, :])
                        bct = mp.tile([64, QCW], F32, tag="bct")
                        nc.gpsimd.partition_broadcast(bct[:], r0[:])
                        bcts.append(bct)
                    qs = slice(qc * QCW, (qc + 1) * QCW)
                    for h in range(2):
                        with nc.allow_low_precision(reason="bf16 ctx"):
                            nc.vector.tensor_mul(
                                ctxt_t[pair][64 * h:64 * (h + 1), qs],
                                ctx_ps[h][0:64, :],
                                bcts[h][:],
                            )
                    if units is None:
                        units = [
                            (lambda s_, d_: lambda: outproj_unit(
                                qc, s_, d_, tail=True,
                                ptag=("proj", "sreg")[d_]))(sub, dd)
                            for sub in range(4) for dd in range(2)
                        ]
                    for u in units:
                        u()

                def new_ctx(pair):
                    return [
                        pp.tile([65, QCW], F32, tag="ctx", bufs=2, name=f"ctx{h}")
                        for h in range(2)
                    ]

                # ---- schedule: deferred-PV pipeline ----
                # Every stretch computes scores+exp for its own (pair, qc)
                # while emitting the PREVIOUS stretch's PV group from stored
                # exp tiles (one per r-slot). PE matmuls therefore never
                # queue behind a just-computed exp, the slab window carries
                # no V/PV work (its PE hump shrinks ~10us), and exactly one
                # PV accumulation group is live at a time, so the 2-bank
                # ctx ring stays strictly sequential.

                def sc_exp(pair, qc, r):
                    sreg = pp.tile([128, 2 * QCW], F32, tag="sreg", bufs=2)
                    expt = ep.tile([128, 2 * QCW], BF16, tag="exp")
                    # scores for the two heads run CONCURRENTLY as K=64
                    # row-group matmuls (tile_position auto-derived from
                    # the 0/64 base partitions of the sliced operands).
                    for h in range(2):
                        hs = slice(64 * h, 64 * (h + 1))
                        nc.tensor.matmul(
                            sreg[:, h * QCW:(h + 1) * QCW],
                            kt_t[pair][hs, r * 128:(r + 1) * 128],
                            qt_t[pair][hs, qc, :],
                            start=True,
                            stop=True,
                        )
                    with nc.allow_low_precision(reason="bf16 exp output"):
                        nc.scalar.activation(expt[:], sreg[:], AF.Exp, scale=0.125)
                    return expt

                def stretch(pair, qc, prev, fillers=None, fill_at=(),
                            pre=None, pre_at=7):
                    ppair, pctx, pexps = prev
                    fill_at = list(fill_at)
                    pre = list(pre) if pre else []
                    exps = []
                    n = 0
                    pv_k = 0
                    for r in range(KT):
                        exps.append((r, sc_exp(pair, qc, r)))
                        n += 1
                        # PV(prev, k) rides slot k+3: the previous group's
                        # c2 bank drain runs at stretch start, so an early
                        # PV would head-of-line block the PE on the
                        # ctx-ring release; the last three PVs bunch at the
                        # final slot.
                        if pv_k < KT and n >= pv_k + 3:
                            pv_emit(ppair, pctx, *pexps[pv_k])
                            pv_k += 1
                        if pre and n >= pre_at:
                            norm_b(*pre.pop(0))
                        while fillers and fill_at and n == fill_at[0]:
                            fill_at.pop(0)
                            fillers.pop(0)()
                    while pv_k < KT:
                        pv_emit(ppair, pctx, *pexps[pv_k])
                        pv_k += 1
                    while fillers and fill_at:  # safety
                        fill_at.pop(0)
                        fillers.pop(0)()
                    return exps

                # slab window: projections + scores/exp of pair-0 qc0 only;
                # v(0..5) woven into slabs 1-3, v(6..15) defer to stretch 1.
                def warm_fill(k):
                    # dependency-free matmuls squeezed between the DMA-gated
                    # early slabs keep the HAM activity window busy so the
                    # PE reaches (and keeps) the 2.4 GHz p-state early.
                    for _ in range(k):
                        wps = pp.tile([64, QCW], F32, tag="proj", bufs=2, name="warm")
                        nc.tensor.matmul(
                            wps[:], ones_b[:, 0:64],
                            warm_in[:], start=True, stop=True,
                        )

                exps_cur = []
                for s in range(QC):
                    kt_proj(0, s)
                    if s == 0:
                        qt_proj(0, 0)
                    for j in range(4):
                        exps_cur.append((4 * s + j, sc_exp(0, 0, 4 * s + j)))
                    if s >= 1:
                        v_proj(2 * (s - 1))
                        v_proj(2 * s - 1)
                        qt_proj(0, s)
                    if s < 2:
                        warm_fill(4)

                def op_units(qc):
                    return [
                        (lambda q, su, d: lambda: outproj_unit(q, su, d))(qc, sub, dd)
                        for sub in range(4)
                        for dd in range(2)
                    ]

                v_late = [(lambda rr: lambda: v_proj(rr))(r) for r in range(6, 16)]

                kq1 = []
                for s in range(QC):
                    st = []
                    kq1.append((lambda s_, st_: lambda: kt_proj_a(1, s_, st_))(s, st))
                    kq1.append((lambda s_, st_: lambda: kt_proj_b(1, s_, st_))(s, st))
                st0 = []
                kq1.append((lambda st_: lambda: qt_proj_a(1, 0, st_))(st0))
                kq1.append((lambda st_: lambda: qt_proj_b(1, 0, st_))(st0))
                qt1_late = []
                for s in range(1, QC):
                    st = []
                    qt1_late.append((lambda s_, st_: lambda: qt_proj_a(1, s_, st_))(s, st))
                    qt1_late.append((lambda s_, st_: lambda: qt_proj_b(1, s_, st_))(s, st))

                region0 = op_units(0)
                region1 = op_units(1)
                plan = [(0, 1), (0, 2), (0, 3), (1, 0), (1, 1), (1, 2), (1, 3)]
                stretch_fills = [
                    (v_late, [1, 2, 3, 4, 5, 6, 7, 8, 9, 10]),
                    (kq1[0:5], [2, 4, 6, 9, 12]),
                    (kq1[5:10], [2, 4, 6, 9, 12]),
                    (qt1_late[0:2], [3, 6]),
                    (qt1_late[2:4], [3, 6]),
                    (qt1_late[4:6] + region0[5:8], [3, 6, 11, 13, 15]),
                    (region0[0:5] + region1[0:2], [1, 3, 5, 7, 9, 11, 13]),
                ]
                pv_src = (0, 0, exps_cur)
                pre = []
                for (pair, qc), (fl, sl) in zip(plan, stretch_fills):
                    ppair, pqc, pexps = pv_src
                    pctx = new_ctx(ppair)
                    exps_new = stretch(pair, qc, (ppair, pctx, pexps),
                                       fillers=fl, fill_at=sl,
                                       pre=pre, pre_at=7)
                    pre = [(ppair, pqc, norm_a(ppair, pqc, pctx)[2])]
                    pv_src = (pair, qc, exps_new)

                # FINAL: PV(1,3) with region-1 leftovers interleaved, the
                # (1,2) norm muls, then the batched tail chain with the
                # region-2 units as PE filler and region-3 last.
                ppair, pqc, pexps = pv_src
                pctx = new_ctx(1)
                osb_pairs = {}

                def f_unit(qc_, k_):
                    sub, dcol = divmod(k_, 2)

                    # Final-phase unit: 4 PSUM slots (proj + the now-idle
                    # sreg banks), casts alternating DVE/ACT, and the two
                    # dcol halves of one sub staged into a single [128, 2,
                    # 512] tile flushed by ONE SP-queue DMA — the ACT queue
                    # must not carry the DMA issues (they serialized the
                    # final phase at ~1.3us/unit).
                    def run():
                        q0 = qc_ * QCW + sub * 128
                        key = (qc_, sub)
                        if key not in osb_pairs:
                            osb_pairs[key] = op.tile(
                                [128, 2, QCW], BF16, tag="osb2", bufs=2,
                                name="osb2",
                            )
                        big = osb_pairs[key]
                        ops = pp.tile([128, QCW], F32,
                                      tag=("proj", "sreg")[k_ % 2], bufs=2,
                                      name="ops")[:]
                        for pair_ in range(2):
                            nc.tensor.matmul(
                                ops,
                                ctxt_t[pair_][:, q0:q0 + 128],
                                wo_t[:, pair_, dcol * 512:(dcol + 1) * 512],
                                start=(pair_ == 0),
                                stop=(pair_ == 1),
                            )
                        with nc.allow_low_precision(reason="bf16 out"):
                            if dcol == 1:
                                nc.scalar.activation(big[:, 1, :], ops, AF.Copy)
                            else:
                                nc.vector.tensor_copy(big[:, 0, :], ops)
                        if dcol == 1:
                            nc.sync.dma_start(
                                out_d[q0:q0 + 128, :], big[:]
                            )
                    return run
                r1_left = [f_unit(1, k) for k in range(2, 8)]
                for e_ in r1_left[0:2]:
                    e_()
                r1_left = r1_left[2:]
                for k, (r, e) in enumerate(pexps):
                    pv_emit(1, pctx, r, e)
                    if k in (3, 6, 9, 12) and r1_left:
                        r1_left.pop(0)()
                norm_b(*pre[0])
                tail_norm_outproj(1, pqc, pctx,
                                  [f_unit(2, k) for k in range(8)],
                                  units=[f_unit(3, k) for k in range(8)])

    nc.compile()
    return nc


def _get_nc(repeat=1):
    key = repeat
    if key not in _CACHE:
        _CACHE[key] = _build(repeat)
    return _CACHE[key]


def _bqm2(bqg):
    out = np.zeros((128, 2), np.float32)
    out[:, 0] = bqg[0:128]
    out[:, 1] = bqg[128:256]
    return out


def _make_in_maps(query_input, Wq, bq, Wk, Wv, Wo):
    bf = ml_dtypes.bfloat16
    x = np.asarray(query_input, dtype=np.float32)
    in_maps = []
    for core in range(NCORES):
        b, g = divmod(core, NCORES // B)
        cs = slice(g * HPC * HD, (g + 1) * HPC * HD)
        in_maps.append({
            "xt": np.ascontiguousarray(x[b].T).astype(bf),
            "wq": np.ascontiguousarray(Wq[:, cs]).astype(bf),
            "wk": np.ascontiguousarray(Wk[:, cs]).astype(bf),
            "wv": np.ascontiguousarray(Wv[:, cs]).astype(bf),
            "wo": np.ascontiguousarray(Wo[cs, :]).astype(bf),
            "bqm2": np.ascontiguousarray(_bqm2(bq[cs])),
        })
    return in_maps


def kernel(query_input, Wq, bq, Wk, bk, Wv, bv, Wo, bo):
    from concourse.bass_utils import run_bass_kernel_spmd

    Wq = np.asarray(Wq, np.float32)
    Wk = np.asarray(Wk, np.float32)
    Wv = np.asarray(Wv, np.float32)
    Wo = np.asarray(Wo, np.float32)
    bq = np.asarray(bq, np.float32)
    bv = np.asarray(bv, np.float32)
    bo = np.asarray(bo, np.float32)

    nc = _get_nc()
    in_maps = _make_in_maps(query_input, Wq, bq, Wk, Wv, Wo)
    res = run_bass_kernel_spmd(nc, in_maps, core_ids=list(range(NCORES)))

    gpc = NCORES // B  # groups per batch
    out = np.zeros((B, S, D), np.float32)
    for core in range(NCORES):
        b = core // gpc
        out[b] += np.asarray(res.results[core]["out_p"], dtype=np.float32)
    # bv correction (exact) + bo, applied once on the full output
    out += (bv @ Wo + bo)[None, None, :]
    return out


# revision 28
# speedup vs baseline: 1.0591x; 1.0591x over previous
"""Self-contained 8-core Trainium2 Bass kernel for MultiHeadAttention.

Problem: B=2, S=2048, D=1024, H=16 heads (hd=64), f32, self-attention
(no mask), eval mode (dropout = identity).

Sharding: data-parallel over B (2) x tensor-parallel over heads (4 groups
of 4 heads) = 8 cores. Each core computes, for its batch b and its 4
heads: Q/K/V projections (column-sliced), attention, and a partial
output projection (row-sliced Wo). Host sums the 4 partials per batch
and adds the (bv @ Wo + bo) correction (bv never enters the kernel:
ctx rows sum probs to 1, so (ctx+bv) @ Wo = ctx @ Wo + bv @ Wo).

Algebraic simplifications (exact):
  - bk dropped: softmax over k is invariant to the per-q constant Q.bk.
  - softmax without max subtraction (scores bounded, exp safe in f32).
  - bq added per-partition (feature) to Q^T after projection.
  - row normalization deferred past P@V (scale ctx, not probs); row sums
    obtained free via an appended ones-column in V.

Design (v9, ~214us HW vs the 235.6us v2 baseline; ACT-exp is the
binding engine at 128 x ~1.01us):
  - scores row-tiled: the two heads of a pair run as CONCURRENT K=64
    matmuls on PE row-groups (0,0)/(64,0) (tile_position auto-derived
    from the 0/64 base partitions), halving scores streaming time vs
    two serial full-array masked matmuls.
  - deferred-PV pipeline: each attention stretch computes scores+exp
    for its own (pair, qc) while emitting the PREVIOUS qc's PV group
    from stored exp tiles (ep pool bufs=14). PE matmuls never queue
    behind a just-computed exp, and exactly one PV accumulation group
    is live at a time so the 2-bank ctx PSUM ring stays sequential.
    The slab window (DMA-paced) carries only kt/qt projections and
    qc0's scores/exp, plus dependency-free warmup matmuls that hold
    the PE HAM activity window open until real work is dense.
  - softmax denominators: both heads' ones-rows are copied onto
    partitions 0/32 of one tile (engine partition bases must be
    32-aligned) and reciprocal'd in ONE call as exp(-ln(x)) on the ACT
    engine. Ln/Exp/Copy are pinned to the natural_log_exp_and_others
    table set (the act-table pass otherwise assigns each function the
    FIRST set containing it and thrashes 2.7us table loads); the DVE
    reciprocal (6.4ns/lane-element, [1,512] = 3.3us) never blocks the
    DVE FIFO this way.
  - norm muls (norm_b) and outproj units are injected into LATER
    stretches at slots placed behind the ~5.5us norm-chain latency so
    the in-order PE queue never parks on a pending dependency (v2 lost
    ~25us + HAM re-throttles to exactly this).
  - weight DMAs ride the ACT HWDGE queue (idle at start), x^T slabs
    the SP queue; final-phase output DMAs pair both 512-column halves
    of a q-block into one [128, 2, 512] staging tile flushed by a
    single SP DMA (ACT must not carry DMA issues, they serialize the
    tail), with outproj PSUM staging alternating between the proj and
    the (by then idle) sreg banks for a 4-deep cast ping-pong.

Layouts per core: x^T [D, S] slabs; K^T/Q^T per head-pair with the two
heads stacked on partitions (64 features each); scores^T per head as
K-chunk[64f, 128k] @ Q^T[64f, 512q] with k-positions on output
partitions; exp on ACT (f32 PSUM -> bf16 SBUF, [128, 1024] covering
both heads); PV accumulates ctx^T[hd+1, q] over r in PSUM with the
ones-row giving the denominator; out projection contracts head dims
with Wo moving.
"""

import sys

sys.path.insert(0, "/opt/trn_rl_repo")

import numpy as np
import ml_dtypes

B, S, D, H, HD = 2, 2048, 1024, 16, 64
HPC = 4  # heads per core
NCORES = 8
DC = D // 128  # 8 contraction chunks
ST = S // 128  # 16 s-tiles
QCW = 512  # q chunk width == slab width
QC = S // QCW  # 4 q chunks == 4 slabs
KT = S // 128  # 16 k tiles

_CACHE = {}


def _build(repeat=1, warmup=12):
    import concourse.bass as bass  # noqa: F401
    import concourse.mybir as mybir
    import concourse.tile as tile
    from concourse import bacc
    from concourse.alu_op_type import AluOpType
    from concourse.library_config import attn as attn_lib

    # The act-table-load pass assigns each activation the FIRST table set
    # containing its function, so a kernel using both Exp and Ln thrashes
    # between exp_and_others and natural_log (2.7us per switch, twice per
    # attention stretch). Strip Exp/Ln from every set except the combined
    # natural_log_exp_and_others (order/indices preserved, so emitted
    # act_func_set_ids still match act_info.json) so both functions
    # resolve to one resident set.
    if not getattr(bacc, "_ln_exp_tables_patched", False):
        _orig_gat = bacc.get_activation_tables

        def _gat(module_arch):
            t = _orig_gat(module_arch)
            AFt = mybir.ActivationFunctionType
            for name, fns in t.items():
                if name != "natural_log_exp_and_others":
                    fns.discard(AFt.Exp)
                    fns.discard(AFt.Ln)
                    fns.discard(AFt.Copy)
            return t

        bacc.get_activation_tables = _gat
        bacc._ln_exp_tables_patched = True

    F32 = mybir.dt.float32
    BF16 = mybir.dt.bfloat16
    AF = mybir.ActivationFunctionType

    nc = bacc.Bacc("TRN2", target_bir_lowering=False, debug=False)

    xt_d = nc.dram_tensor("xt", [D, S], BF16, kind="ExternalInput")
    wq_d = nc.dram_tensor("wq", [D, HPC * HD], BF16, kind="ExternalInput")
    wk_d = nc.dram_tensor("wk", [D, HPC * HD], BF16, kind="ExternalInput")
    wv_d = nc.dram_tensor("wv", [D, HPC * HD], BF16, kind="ExternalInput")
    wo_d = nc.dram_tensor("wo", [HPC * HD, D], BF16, kind="ExternalInput")
    bqm_d = nc.dram_tensor("bqm2", [128, 2], F32, kind="ExternalInput")
    out_d = nc.dram_tensor("out_p", [S, D], BF16, kind="ExternalOutput")

    with tile.TileContext(nc) as tc:
        nc.gpsimd.load_library(attn_lib)
        with (
            tc.tile_pool(name="wp", bufs=1) as wp,
            tc.tile_pool(name="xp", bufs=1) as xp,
            tc.tile_pool(name="qk", bufs=1) as qk,
            tc.tile_pool(name="vp", bufs=1) as vp,
            tc.tile_pool(name="ep", bufs=18) as ep,
            tc.tile_pool(name="cp", bufs=1) as cp,
            tc.tile_pool(name="c2", bufs=4) as c2p,
            tc.tile_pool(name="mp", bufs=2) as mp,
            tc.tile_pool(name="op", bufs=4) as op,
            tc.tile_pool(name="pp", bufs=2, space="PSUM") as pp,
        ):
            # persistent tiles (loaded / initialized once, reused each rep)
            wk_t = wp.tile([128, DC, HPC * HD], BF16, tag="wk")
            wq_t = wp.tile([128, DC, HPC * HD], BF16, tag="wq")
            wv_t = wp.tile([128, DC, HPC * HD], BF16, tag="wv")
            wo_t = wp.tile([128, 2, D], BF16, tag="wo")
            bqm_t = wp.tile([128, 2], F32, tag="bqm")
            ones_b = wp.tile([128, 64], BF16, tag="ones")
            warm_in = wp.tile([128, QCW], BF16, tag="warmin")
            xt_t = xp.tile([128, DC, S], BF16, tag="xt")

            # DMA order: wk+wq then slab0 gate the first projections.
            # Weights ride the ACT HWDGE queue (idle at start) so they don't
            # serialize behind the x^T slab stream on the SP queue.
            nc.scalar.dma_start(wk_t[:], wk_d.rearrange("(c p) n -> p c n", p=128))
            nc.scalar.dma_start(wq_t[:], wq_d.rearrange("(c p) n -> p c n", p=128))
            nc.scalar.dma_start(bqm_t[:], bqm_d[:])
            nc.vector.memset(ones_b[:], 1.0)
            nc.vector.memset(warm_in[:], 0.0)

            import contextlib

            def emit_input_dmas():
                # ---- x^T slab DMAs (+ wv after slab0, wo after slab2)
                for s in range(QC):
                    qs = slice(s * QCW, (s + 1) * QCW)
                    for c in range(DC):
                        nc.sync.dma_start(
                            xt_t[:, c, qs], xt_d[c * 128:(c + 1) * 128, qs]
                        )
                    if s == 0:
                        nc.scalar.dma_start(
                            wv_t[:], wv_d.rearrange("(c p) n -> p c n", p=128)
                        )
                    elif s == 2:
                        nc.scalar.dma_start(
                            wo_t[:], wo_d.rearrange("(c p) n -> p c n", p=128)
                        )

            if repeat > 1:
                # In-loop DMAs go through software descriptor generation and
                # dominate the loop body, so hoist them for timing builds.
                emit_input_dmas()
                _engs = [mybir.EngineType.PE, mybir.EngineType.Activation,
                         mybir.EngineType.DVE, mybir.EngineType.SP,
                         mybir.EngineType.Pool]
                rep_ctx = tc.For_i(0, repeat, hint_engines=_engs, staggered_reset=True)
            else:
                rep_ctx = contextlib.nullcontext()
            with rep_ctx:
                if repeat == 1:
                    emit_input_dmas()

                # ---- per-rep tiles
                v1_t = vp.tile([128, ST, HPC * 65], BF16, tag="v1")
                with nc.allow_low_precision(reason="bf16 operands"):
                    nc.vector.tensor_copy(
                        v1_t[:].rearrange("p s (h c) -> p s h c", c=65)[:, :, :, 64],
                        ones_b[:, 0:64].rearrange("p (s h) -> p s h", s=ST),
                    )
                kt_t = [qk.tile([128, S], BF16, tag=f"kt{p}", name=f"kt{p}") for p in range(2)]
                qt_t = [qk.tile([128, QC, QCW], BF16, tag=f"qt{p}", name=f"qt{p}") for p in range(2)]
                ctxt_t = [cp.tile([128, S], BF16, tag=f"ct{p}", name=f"ct{p}") for p in range(2)]

                # ---- PE warmup: ramp the p-state before real work arrives.
                for w in range(warmup):
                    wps = pp.tile([64, QCW], F32, tag="proj", bufs=2, name="warm")
                    nc.tensor.matmul(
                        wps[:], ones_b[:, 0:64],
                        warm_in[:], start=True, stop=True,
                    )

                # ---- projection units
                def kt_proj_a(pair, s, state):
                    qs = slice(s * QCW, (s + 1) * QCW)
                    kps = pp.tile([128, QCW], F32, tag="proj", bufs=2, name="kps")
                    state.append(kps)
                    for c in range(DC // 2):
                        nc.tensor.matmul(
                            kps[:],
                            wk_t[:, c, pair * 128:(pair + 1) * 128],
                            xt_t[:, c, qs],
                            start=(c == 0),
                            stop=False,
                        )

                def kt_proj_b(pair, s, state):
                    qs = slice(s * QCW, (s + 1) * QCW)
                    kps = state.pop()
                    for c in range(DC // 2, DC):
                        nc.tensor.matmul(
                            kps[:],
                            wk_t[:, c, pair * 128:(pair + 1) * 128],
                            xt_t[:, c, qs],
                            start=False,
                            stop=(c == DC - 1),
                        )
                    with nc.allow_low_precision(reason="bf16 operands"):
                        nc.vector.tensor_copy(kt_t[pair][:, qs], kps[:])

                def kt_proj(pair, s):
                    st = []
                    kt_proj_a(pair, s, st)
                    kt_proj_b(pair, s, st)

                def qt_proj_a(pair, s, state):
                    qs = slice(s * QCW, (s + 1) * QCW)
                    qps = pp.tile([128, QCW], F32, tag="proj", bufs=2, name="qps")
                    state.append(qps)
                    for c in range(DC // 2):
                        nc.tensor.matmul(
                            qps[:],
                            wq_t[:, c, pair * 128:(pair + 1) * 128],
                            xt_t[:, c, qs],
                            start=(c == 0),
                            stop=False,
                        )

                def qt_proj_b(pair, s, state):
                    qs = slice(s * QCW, (s + 1) * QCW)
                    qps = state.pop()
                    for c in range(DC // 2, DC):
                        nc.tensor.matmul(
                            qps[:],
                            wq_t[:, c, pair * 128:(pair + 1) * 128],
                            xt_t[:, c, qs],
                            start=False,
                            stop=(c == DC - 1),
                        )
                    # bq added per-partition (feature dim) — heads need no
                    # masking since scores contract only their own 64 rows.
                    with nc.allow_low_precision(reason="bf16 operands"):
                        nc.vector.tensor_scalar_add(
                            qt_t[pair][:, s, :], qps[:],
                            bqm_t[:, pair:pair + 1],
                        )

                def qt_proj(pair, s):
                    st = []
                    qt_proj_a(pair, s, st)
                    qt_proj_b(pair, s, st)

                def v_proj(st):
                    vps = pp.tile([128, HPC * HD], F32, tag="proj", bufs=2, name="vps")
                    for c in range(DC):
                        nc.tensor.matmul(
                            vps[:],
                            xt_t[:, c, st * 128:(st + 1) * 128],
                            wv_t[:, c, :],
                            start=(c == 0),
                            stop=(c == DC - 1),
                        )
                    with nc.allow_low_precision(reason="bf16 operands"):
                        nc.vector.tensor_copy(
                            v1_t[:, st, :].rearrange("p (h c) -> p h c", c=65)[:, :, 0:64],
                            vps[:].rearrange("p (h c) -> p h c", c=64),
                        )

                # ---- attention pieces
                def pv_emit(pair, ctx_ps, r, expt):
                    for h in range(2):
                        hh = 2 * pair + h
                        nc.tensor.matmul(
                            ctx_ps[h][:],
                            v1_t[:, r, 65 * hh:65 * hh + 65],
                            expt[:, h * QCW:(h + 1) * QCW],
                            start=(r == 0),
                            stop=(r == KT - 1),
                        )

                def attn_rs(pair, qc, rs, ctx_ps, fillers=None, fill_at=(),
                            pre=None, pre_at=7, pend=None, flush=True,
                            collect=None):
                    # `pend` carries the not-yet-emitted PV of the previous r
                    # (possibly across the slab-window segments of one call).
                    # `collect` (a list) switches to scores+exp only: the
                    # (r, expt) pairs are stashed so their PVs can run later
                    # as fillers (used to thin the slab-window phase).
                    # `pre` is a list of deferred normalization-mul handle
                    # sets; one is consumed per r starting at n == pre_at.
                    fill_at = list(fill_at)
                    pre = list(pre) if pre else []
                    n = 0
                    for r in rs:
                        sreg = pp.tile([128, 2 * QCW], F32, tag="sreg", bufs=2)
                        expt = ep.tile([128, 2 * QCW], BF16, tag="exp")
                        # scores for the two heads run CONCURRENTLY as K=64
                        # row-group matmuls (tile_position auto-derived from
                        # the 0/64 base partitions of the sliced operands).
                        for h in range(2):
                            hs = slice(64 * h, 64 * (h + 1))
                            nc.tensor.matmul(
                                sreg[:, h * QCW:(h + 1) * QCW],
                                kt_t[pair][hs, r * 128:(r + 1) * 128],
                                qt_t[pair][hs, qc, :],
                                start=True,
                                stop=True,
                            )
                        with nc.allow_low_precision(reason="bf16 exp output"):
                            nc.scalar.activation(expt[:], sreg[:], AF.Exp, scale=0.125)
                        if collect is not None:
                            collect.append((r, expt))
                        else:
                            # emit the PREVIOUS r's PV only now: the next
                            # scores matmul never sits behind a PV that is
                            # waiting on exp, so exp is never starved.
                            if pend is not None:
                                pv_emit(pair, ctx_ps, *pend)
                            pend = (r, expt)
                        n += 1
                        if pre and pre_at <= n < pre_at + len(pre) + 16:
                            if n >= pre_at:
                                norm_b(*pre.pop(0))
                        while fillers and fill_at and n == fill_at[0]:
                            fill_at.pop(0)
                            fillers.pop(0)()
                    if flush and pend is not None:
                        pv_emit(pair, ctx_ps, *pend)
                        pend = None
                    return pend

                def norm_a1(pair, qc, ctx_ps):
                    # Boundary part of the norm chain: drain the ctx PSUM
                    # banks (they have no double buffer — the next PV group
                    # waits on their release) and stage both heads'
                    # denominators onto partitions 0 and 32 of one tile
                    # (engine partition bases must be 32-aligned). The ACT
                    # part (norm_a2) is deferred into the next stretch at
                    # slot 2: inserting Ln/Exp between exp(r15) and
                    # exp(r0') costs the boundary twice (the 2-buffer sreg
                    # ring echoes the ACT hiccup into the next scores).
                    handles = []
                    d2 = mp.tile([33, QCW], F32, tag="d2")
                    for h in range(2):
                        c2 = c2p.tile([65, QCW], F32, tag="c2s", name=f"c2s{h}")
                        nc.vector.tensor_copy(c2[:], ctx_ps[h][:])
                        nc.vector.tensor_copy(
                            d2[32 * h:32 * h + 1, :], c2[64:65, :]
                        )
                        handles.append(c2)
                    holder = []

                    def chain2():
                        # reciprocal via exp(-ln(x)) on ACT: Ln and Exp
                        # share one table set, and the DVE FIFO is never
                        # blocked by the 3.3us DVE reciprocal.
                        lg = mp.tile([33, QCW], F32, tag="lg")
                        nc.scalar.activation(lg[:], d2[:], AF.Ln)
                        dr = mp.tile([33, QCW], F32, tag="dr")
                        nc.scalar.activation(dr[:], lg[:], AF.Exp, scale=-1.0)
                        for h in range(2):
                            r0 = mp.tile([1, QCW], F32, tag="rsum")
                            nc.vector.tensor_copy(
                                r0[:], dr[32 * h:32 * h + 1, :]
                            )
                            bct = mp.tile([64, QCW], F32, tag="bc")
                            nc.gpsimd.partition_broadcast(bct[:], r0[:])
                            holder.append((handles[h], bct))
                    return (pair, qc, holder, chain2)

                def norm_b(pair, qc, holder, chain2=None):
                    qs = slice(qc * QCW, (qc + 1) * QCW)
                    for h in range(2):
                        c2, bct = holder[h]
                        with nc.allow_low_precision(reason="bf16 ctx"):
                            nc.vector.tensor_mul(
                                ctxt_t[pair][64 * h:64 * (h + 1), qs],
                                c2[0:64, :],
                                bct[:],
                            )

                def outproj_unit(qc, sub, dcol, tail=False, ptag="proj"):
                    q0 = qc * QCW + sub * 128
                    ops = pp.tile([128, QCW], F32, tag=ptag, bufs=2, name="ops")[:]
                    for pair in range(2):
                        nc.tensor.matmul(
                            ops,
                            ctxt_t[pair][:, q0:q0 + 128],
                            wo_t[:, pair, dcol * 512:(dcol + 1) * 512],
                            start=(pair == 0),
                            stop=(pair == 1),
                        )
                    osb = op.tile([128, QCW], BF16, tag="osb")
                    with nc.allow_low_precision(reason="bf16 out"):
                        if tail and dcol == 1:
                            # ACT is idle in the tail: alternating the PSUM
                            # drain between DVE and ACT halves the cast pace
                            # that gates the 2-slot proj-PSUM ping-pong.
                            # (AF Copy lives in every table set: no switch.)
                            nc.scalar.activation(osb[:], ops, AF.Copy)
                        else:
                            nc.vector.tensor_copy(osb[:], ops)
                    eng = nc.scalar if tail else nc.sync
                    eng.dma_start(
                        out_d[q0:q0 + 128, dcol * 512:(dcol + 1) * 512], osb[:]
                    )

                def tail_norm_outproj(pair, qc, ctx_ps, extra=(), units=None):
                    # last-call variant: one batched norm chain over all 512
                    # columns (denominators straight from PSUM, reciprocal
                    # on ACT which is idle by now), with held-back outproj
                    # units as PE filler while the chain resolves, then all
                    # 8 output-projection units. Tail DMAs ride the ACT
                    # HWDGE queue (idle) to halve the final drain.
                    extra = list(extra)
                    d2c = mp.tile([33, QCW], F32, tag="d2t")
                    for h in range(2):
                        nc.vector.tensor_copy(
                            d2c[32 * h:32 * h + 1, :], ctx_ps[h][64:65, :]
                        )
                    lgc = mp.tile([33, QCW], F32, tag="lgt")
                    nc.scalar.activation(lgc[:], d2c[:], AF.Ln)
                    drc = mp.tile([33, QCW], F32, tag="drt")
                    nc.scalar.activation(drc[:], lgc[:], AF.Exp, scale=-1.0)
                    while extra:  # PE filler while the chain resolves
                        extra.pop(0)()
                    bcts = []
                    for h in range(2):
                        r0 = mp.tile([1, QCW], F32, tag="rsumt")
                        nc.vector.tensor_copy(r0[:], drc[32 * h:32 * h + 1, :])
                        bct = mp.tile([64, QCW], F32, tag="bct")
                        nc.gpsimd.partition_broadcast(bct[:], r0[:])
                        bcts.append(bct)
                    qs = slice(qc * QCW, (qc + 1) * QCW)
                    for h in range(2):
                        with nc.allow_low_precision(reason="bf16 ctx"):
                            nc.vector.tensor_mul(
                                ctxt_t[pair][64 * h:64 * (h + 1), qs],
                                ctx_ps[h][0:64, :],
                                bcts[h][:],
                            )
                    if units is None:
                        units = [
                            (lambda s_, d_: lambda: outproj_unit(
                                qc, s_, d_, tail=True,
                                ptag=("proj", "sreg")[d_]))(sub, dd)
                            for sub in range(4) for dd in range(2)
                        ]
                    for u in units:
                        u()

                def new_ctx(pair):
                    return [
                        pp.tile([65, QCW], F32, tag="ctx", bufs=2, name=f"ctx{h}")
                        for h in range(2)
                    ]

                # ---- schedule: deferred-PV pipeline ----
                # Every stretch computes scores+exp for its own (pair, qc)
                # while emitting the PREVIOUS stretch's PV group from stored
                # exp tiles (one per r-slot). PE matmuls therefore never
                # queue behind a just-computed exp, the slab window carries
                # no V/PV work (its PE hump shrinks ~10us), and exactly one
                # PV accumulation group is live at a time, so the 2-bank
                # ctx ring stays strictly sequential.

                def sc_exp(pair, qc, r):
                    sreg = pp.tile([128, 2 * QCW], F32, tag="sreg", bufs=2)
                    expt = ep.tile([128, 2 * QCW], BF16, tag="exp")
                    # scores for the two heads run CONCURRENTLY as K=64
                    # row-group matmuls (tile_position auto-derived from
                    # the 0/64 base partitions of the sliced operands).
                    for h in range(2):
                        hs = slice(64 * h, 64 * (h + 1))
                        nc.tensor.matmul(
                            sreg[:, h * QCW:(h + 1) * QCW],
                            kt_t[pair][hs, r * 128:(r + 1) * 128],
                            qt_t[pair][hs, qc, :],
                            start=True,
                            stop=True,
                        )
                    with nc.allow_low_precision(reason="bf16 exp output"):
                        nc.scalar.activation(expt[:], sreg[:], AF.Exp, scale=0.125)
                    return expt

                def stretch(pair, qc, prev, fillers=None, fill_at=(),
                            pre=None, pre_at=7):
                    ppair, pctx, pexps = prev
                    fill_at = list(fill_at)
                    pre = list(pre) if pre else []
                    exps = []
                    n = 0
                    pv_k = 0
                    for r in range(KT):
                        exps.append((r, sc_exp(pair, qc, r)))
                        n += 1
                        if n == 2:
                            for p_ in pre:
                                p_[3]()  # deferred ACT recip chain
                        # PV(prev, k) schedule: r0 waits until slot 2 (the
                        # previous group's c2 bank drain needs ~1.4us at
                        # stretch start), r1..r11 ride slot k+1, and the
                        # last four double up on slots 12-14 so the group
                        # STOPS a couple of slots before the boundary: its
                        # own c2 drain then completes before the next
                        # stretch needs the banks, and the next stretch's
                        # first scores are never queued behind a PV burst.
                        _pv_slot = (2, 2, 3, 4, 5, 6, 7, 8, 9, 10, 11, 12,
                                    12, 13, 13, 14)
                        while pv_k < KT and n >= _pv_slot[pv_k]:
                            pv_emit(ppair, pctx, *pexps[pv_k])
                            pv_k += 1
                        if pre and n >= pre_at:
                            norm_b(*pre.pop(0))
                        while fillers and fill_at and n == fill_at[0]:
                            fill_at.pop(0)
                            fillers.pop(0)()
                    while pv_k < KT:
                        pv_emit(ppair, pctx, *pexps[pv_k])
                        pv_k += 1
                    while fillers and fill_at:  # safety
                        fill_at.pop(0)
                        fillers.pop(0)()
                    return exps

                # slab window: projections + scores/exp of pair-0 qc0 only;
                # v(0..5) woven into slabs 1-3, v(6..15) defer to stretch 1.
                def warm_fill(k):
                    # dependency-free matmuls squeezed between the DMA-gated
                    # early slabs keep the HAM activity window busy so the
                    # PE reaches (and keeps) the 2.4 GHz p-state early.
                    for _ in range(k):
                        wps = pp.tile([64, QCW], F32, tag="proj", bufs=2, name="warm")
                        nc.tensor.matmul(
                            wps[:], ones_b[:, 0:64],
                            warm_in[:], start=True, stop=True,
                        )

                exps_cur = []
                for s in range(QC):
                    kt_proj(0, s)
                    if s == 0:
                        qt_proj(0, 0)
                    for j in range(4):
                        exps_cur.append((4 * s + j, sc_exp(0, 0, 4 * s + j)))
                    if s >= 1:
                        v_proj(2 * (s - 1))
                        v_proj(2 * s - 1)
                        if s < 3:
                            # qt(0,3) is only needed at stretch S3: it
                            # leaves the PE-bound slab window and rides
                            # S2's spare filler slots instead.
                            qt_proj(0, s)
                    if s < 2:
                        warm_fill(4)

                def op_units(qc):
                    return [
                        (lambda q, su, d: lambda: outproj_unit(q, su, d))(qc, sub, dd)
                        for sub in range(4)
                        for dd in range(2)
                    ]

                v_late = [(lambda rr: lambda: v_proj(rr))(r) for r in range(6, 16)]

                kq1 = []
                for s in range(QC):
                    st = []
                    kq1.append((lambda s_, st_: lambda: kt_proj_a(1, s_, st_))(s, st))
                    kq1.append((lambda s_, st_: lambda: kt_proj_b(1, s_, st_))(s, st))
                st0 = []
                kq1.append((lambda st_: lambda: qt_proj_a(1, 0, st_))(st0))
                kq1.append((lambda st_: lambda: qt_proj_b(1, 0, st_))(st0))
                qt1_late = []
                for s in range(1, QC):
                    st = []
                    qt1_late.append((lambda s_, st_: lambda: qt_proj_a(1, s_, st_))(s, st))
                    qt1_late.append((lambda s_, st_: lambda: qt_proj_b(1, s_, st_))(s, st))

                region0 = op_units(0)
                region1 = op_units(1)
                plan = [(0, 1), (0, 2), (0, 3), (1, 0), (1, 1), (1, 2), (1, 3)]
                stretch_fills = [
                    (v_late, [1, 2, 3, 4, 5, 6, 7, 8, 9, 10]),
                    (kq1[0:5] + [lambda: qt_proj(0, 3)], [2, 4, 6, 9, 12, 14]),
                    (kq1[5:10], [2, 4, 6, 9, 12]),
                    (qt1_late[0:2], [3, 6]),
                    (qt1_late[2:4], [3, 6]),
                    (qt1_late[4:6] + region0[5:8], [3, 6, 11, 13, 15]),
                    (region0[0:5] + region1[0:2], [1, 3, 5, 7, 9, 11, 13]),
                ]
                pv_src = (0, 0, exps_cur)
                pre = []
                na_box = []
                for idx, ((pair, qc), (fl, sl)) in enumerate(zip(plan, stretch_fills)):
                    ppair, pqc, pexps = pv_src
                    pctx = new_ctx(ppair)
                    if idx == len(plan) - 1:
                        # the PV-prev group stops at slot 14, so the last
                        # stretch can emit its norm chain at slot 15 —
                        # the FINAL phase's region-2 outproj units then
                        # have their muls ~4us sooner.
                        def _last_na(pp_=ppair, pq_=pqc, px_=pctx):
                            na_ = norm_a1(pp_, pq_, px_)
                            na_[3]()
                            na_box.append((na_[0], na_[1], na_[2], lambda: None))
                        fl = list(fl) + [_last_na]
                        sl = list(sl) + [15]
                    exps_new = stretch(pair, qc, (ppair, pctx, pexps),
                                       fillers=fl, fill_at=sl,
                                       pre=pre, pre_at=7)
                    if idx == len(plan) - 1:
                        pre = [na_box[0]]
                    else:
                        # the ACT recip chain (na[3]) is deferred to slot 2
                        # of the next stretch: inserting Ln/Exp between
                        # exp(r15) and exp(r0') costs the boundary twice
                        # (the 2-buffer sreg ring echoes the ACT hiccup
                        # into the next scores).
                        pre = [norm_a1(ppair, pqc, pctx)]
                    pv_src = (pair, qc, exps_new)

                # FINAL: PV(1,3) with region-1 leftovers interleaved, the
                # (1,2) norm muls, then the batched tail chain with the
                # region-2 units as PE filler and region-3 last.
                ppair, pqc, pexps = pv_src
                pctx = new_ctx(1)
                osb_pairs = {}

                def f_unit(qc_, k_):
                    sub, dcol = divmod(k_, 2)

                    # Final-phase unit: 4 PSUM slots (proj + the now-idle
                    # sreg banks), casts alternating DVE/ACT, and the two
                    # dcol halves of one sub staged into a single [128, 2,
                    # 512] tile flushed by ONE SP-queue DMA — the ACT queue
                    # must not carry the DMA issues (they serialized the
                    # final phase at ~1.3us/unit).
                    def run():
                        q0 = qc_ * QCW + sub * 128
                        key = (qc_, sub)
                        if key not in osb_pairs:
                            osb_pairs[key] = op.tile(
                                [128, 2, QCW], BF16, tag="osb2", bufs=2,
                                name="osb2",
                            )
                        big = osb_pairs[key]
                        ops = pp.tile([128, QCW], F32,
                                      tag=("proj", "sreg")[k_ % 2], bufs=2,
                                      name="ops")[:]
                        for pair_ in range(2):
                            nc.tensor.matmul(
                                ops,
                                ctxt_t[pair_][:, q0:q0 + 128],
                                wo_t[:, pair_, dcol * 512:(dcol + 1) * 512],
                                start=(pair_ == 0),
                                stop=(pair_ == 1),
                            )
                        with nc.allow_low_precision(reason="bf16 out"):
                            if dcol == 1:
                                nc.scalar.activation(big[:, 1, :], ops, AF.Copy)
                            else:
                                nc.vector.tensor_copy(big[:, 0, :], ops)
                        if dcol == 1:
                            # alternate the paired flushes between the SP
                            # and ACT HWDGE queues: the final drain was
                            # bandwidth-limited on one queue (~6us gap).
                            eng = nc.sync if sub % 2 == 0 else nc.scalar
                            eng.dma_start(
                                out_d[q0:q0 + 128, :], big[:]
                            )
                    return run
                r1_left = [f_unit(1, k) for k in range(2, 8)]
                for e_ in r1_left[0:2]:
                    e_()
                r1_left = r1_left[2:]
                for k, (r, e) in enumerate(pexps):
                    pv_emit(1, pctx, r, e)
                    if k in (3, 6, 9, 12) and r1_left:
                        r1_left.pop(0)()
                norm_b(*pre[0])
                tail_norm_outproj(1, pqc, pctx,
                                  [f_unit(2, k) for k in range(8)],
                                  units=[f_unit(3, k) for k in range(8)])

    nc.compile()
    return nc


def _get_nc(repeat=1):
    key = repeat
    if key not in _CACHE:
        _CACHE[key] = _build(repeat)
    return _CACHE[key]


def _bqm2(bqg):
    out = np.zeros((128, 2), np.float32)
    out[:, 0] = bqg[0:128]
    out[:, 1] = bqg[128:256]
    return out


def _make_in_maps(query_input, Wq, bq, Wk, Wv, Wo):
    bf = ml_dtypes.bfloat16
    x = np.asarray(query_input, dtype=np.float32)
    in_maps = []
    for core in range(NCORES):
        b, g = divmod(core, NCORES // B)
        cs = slice(g * HPC * HD, (g + 1) * HPC * HD)
        in_maps.append({
            "xt": np.ascontiguousarray(x[b].T).astype(bf),
            "wq": np.ascontiguousarray(Wq[:, cs]).astype(bf),
            "wk": np.ascontiguousarray(Wk[:, cs]).astype(bf),
            "wv": np.ascontiguousarray(Wv[:, cs]).astype(bf),
            "wo": np.ascontiguousarray(Wo[cs, :]).astype(bf),
            "bqm2": np.ascontiguousarray(_bqm2(bq[cs])),
        })
    return in_maps


def kernel(query_input, Wq, bq, Wk, bk, Wv, bv, Wo, bo):
    from concourse.bass_utils import run_bass_kernel_spmd

    Wq = np.asarray(Wq, np.float32)
    Wk = np.asarray(Wk, np.float32)
    Wv = np.asarray(Wv, np.float32)
    Wo = np.asarray(Wo, np.float32)
    bq = np.asarray(bq, np.float32)
    bv = np.asarray(bv, np.float32)
    bo = np.asarray(bo, np.float32)

    nc = _get_nc()
    in_maps = _make_in_maps(query_input, Wq, bq, Wk, Wv, Wo)
    res = run_bass_kernel_spmd(nc, in_maps, core_ids=list(range(NCORES)))

    gpc = NCORES // B  # groups per batch
    out = np.zeros((B, S, D), np.float32)
    for core in range(NCORES):
        b = core // gpc
        out[b] += np.asarray(res.results[core]["out_p"], dtype=np.float32)
    # bv correction (exact) + bo, applied once on the full output
    out += (bv @ Wo + bo)[None, None, :]
    return out
